# revision 48
# baseline (speedup 1.0000x reference)
"""Trainium2 Bass kernel for nn_BitSwapWrapper.

Reference computation:
    g    = x[rows, idx]                       # one gathered element per row
    u    = coeff * (bitflip(g, bit_pos) - g)
    pert = scatter(zeros_like(x), (rows, idx), u)
    out  = (x + pert) @ W + b

Because pert has exactly one nonzero per row, (x + pert) @ W decomposes as
    out[i, :] = (x @ W)[i, :] + u[i] * W[idx[i], :] + b
so no [B, F] scatter tensor is ever materialized: the kernel streams x
through a K-accumulated matmul and applies the rank-per-row correction with
an indirect-DMA gather of the needed W rows (u is formed on-chip from the
exact fp32 gathered values + bit positions).

Distribution: data-parallel over the batch dim across 8 NeuronCores
(x/idx/bit_positions sharded on dim 0, W/b/coeff replicated), per the
sharding hint. Each core computes its [512, 256] slice of the output.

Precision/traffic design (memory-regime problem, PE sets the floor):
  - x is streamed as int8 with per-batch-row scales (symmetric absmax
    quantization done on host). 8 MB/core instead of 32 MB fp32.
  - W is streamed as bf16 (8 MB/core).
  - int8 x is converted to bf16 on-chip (exact: |q| <= 127), with the
    conversion alternated between the DVE and Activation engines so it
    stays off the critical path; the PE then runs bf16 x bf16 matmuls
    (~55 us of PE work, the binding resource -- DMA is ~46 us).
  - WSTAT form: W-half tiles are the stationary operand and x chunks the
    moving one (N=512), halving the PE instruction count vs the x-stationary
    form; measured ~3.5 us faster on HW (per-instruction overhead the cost
    model does not carry). Output leaves as [O, BC], transposed on host.
  - the bit-flip correction is folded into PSUM with one diag(u) matmul per
    (row-block, O-half); per-row scales apply in the epilogue as one
    [P, BC] multiply against a PE-broadcast scale tile (built once, outside
    the rep loop, since it is rep-invariant).
  - the correction uses exact fp32 gathered values (gh), so the output
    error is x-quantization + W rounding only.
  - hybrid K-split: the last 16 k-chunks (12.5% of K) run as fp8e4
    DoubleRow matmuls (2 chunks per instruction at 0.5 cycles/row),
    cutting the PE floor from 54.6 us to 49.5 us. x is e4m3(x/(64 s)) and
    W is e4m3(64 W) there, so products land in the same units as the int8
    part and accumulate into the same PSUM group. End-to-end error
    1.52e-2 vs the 2e-2 gate, deterministic (fixed inputs), HW-verified.
"""

import numpy as np

import concourse.bass as bass
import concourse.mybir as mybir
from concourse.bass_utils import run_bass_kernel_spmd
from concourse.tile import TileContext

N_CORES = 8
B, F, O = 4096, 16384, 256
BC = B // N_CORES        # 512 batch rows per core
P = 128
KC = F // P              # 128 contraction chunks
MB = BC // P             # 4 output row-blocks per core

F32 = mybir.dt.float32
BF16 = mybir.dt.bfloat16
I32 = mybir.dt.int32
I8 = mybir.dt.int8
FP8 = mybir.dt.float8e4  # = ml_dtypes.float8_e4m3 on host

# Hybrid K-split: the last 2*FP8_PAIRS k-chunks run as fp8 DoubleRow matmuls
# (2 chunks per matmul at 0.5 cycles/row -> ~5.1 us less PE work per rep).
# Error budget: int8-x part ~9.6e-3, fp8 part adds in quadrature ->
# 1.52e-2 measured end-to-end on the real inputs (gate 2e-2). W in the fp8
# region is pre-scaled by 64 (keeps e4m3 out of subnormals); the 1/64 is
# undone in the epilogue via a separate PSUM accumulator.
FP8_PAIRS = 10
FP8_WSCALE = 64.0


def _split_multi_waits(nc):
    """This container's walrus build rejects more than one sync-wait command
    per instruction; split extras onto single-wait NOPs on the same engine."""
    cur_bb = nc.cur_bb.bb
    for f in nc.m.functions:
        for bb in f.blocks:
            il = bb.instructions
            i = 0
            while i < len(il):
                ins = il[i]
                si = getattr(ins, "sync_info", None)
                if si is not None and si.on_wait and len(si.on_wait) > 1:
                    waits = list(si.on_wait)
                    extra, keep = waits[:-1], waits[-1:]
                    carriers = []
                    for w in extra:
                        nop = nc.engines[ins.engine].nop(nofuse=True).ins
                        tail = cur_bb.instructions.pop()
                        assert tail is nop
                        nop.sync_info = mybir.SyncInfo(on_wait=[w], on_update=[])
                        carriers.append(nop)
                    ins.sync_info = mybir.SyncInfo(
                        on_wait=keep, on_update=list(si.on_update or [])
                    )
                    il[i:i] = carriers
                    i += len(carriers)
                i += 1


def _slab_plan(cpg, prime, kc_total=KC):
    """Chunk slabs: a few small ones first to prime the PE pipeline, then
    full-size slabs. Returns [(k0, nchunks), ...] covering kc_total chunks."""
    slabs = []
    k = 0
    for n in prime:
        slabs.append((k, n))
        k += n
    while k < kc_total:
        n = min(cpg, kc_total - k)
        slabs.append((k, n))
        k += n
    return slabs


WSTAT = True  # stationary-W matmul form: half the PE instructions (N=512)


def build(reps=1, stream_bufs=12, cpg=4, prime=(1, 1, 2), with_bias=True,
          act_slots=(1,), cast_mod=2, prep_at=(6, 9, 12, 15), prep_dma_at=2,
          wstat=None, fp8_pairs=None):
    if wstat is None:
        wstat = WSTAT
    if wstat and with_bias:
        wstat = False  # wstat path assumes b == 0
    if fp8_pairs is None:
        fp8_pairs = FP8_PAIRS
    if not wstat:
        fp8_pairs = 0
    kp = fp8_pairs
    kc2 = KC - 2 * kp  # chunks handled by the int8->bf16 stream
    nc = bass.Bass("TRN2", target_bir_lowering=False, debug=False)
    xq = nc.dram_tensor("xq", [P, KC * BC], I8, kind="ExternalInput").ap()
    wq = nc.dram_tensor("wq", [P, KC * O], BF16, kind="ExternalInput").ap()
    wf = nc.dram_tensor("wf", [F, O], F32, kind="ExternalInput").ap()
    # packed per-row scalars: [idx | bpos | gh bits | scale bits], MB cols each
    prep = nc.dram_tensor("prep", [P, 4 * MB], I32, kind="ExternalInput").ap()
    bb_ = nc.dram_tensor("b", [O], BF16, kind="ExternalInput").ap()
    coeff = nc.dram_tensor("coeff", [P, 1], F32, kind="ExternalInput").ap()
    if wstat:
        srow = nc.dram_tensor("srow", [1, BC], F32, kind="ExternalInput").ap()
        # row-major bf16 W for the correction gather: bf16 operands keep the
        # diag(u) matmuls at 1 cycle/row (f32 costs 4x on the PE)
        wb16 = nc.dram_tensor("wb16", [F, O], BF16, kind="ExternalInput").ap()
        out = nc.dram_tensor("out", [O, BC], F32, kind="ExternalOutput").ap()
    else:
        out = nc.dram_tensor("out", [BC, O], F32, kind="ExternalOutput").ap()
    if kp:
        # fp8 tail region, plane-major pair layouts for DoubleRow:
        # xf8[p, c*2*BC + j*BC + n] = e4m3(x[n, (kc2+2c+j)*P + p] / s[n])
        # wf8[p, c*4*P*? ...]: [c, h, j, m] -> W8[(kc2+2c+j)*P + p, h*P+m]
        xf8 = nc.dram_tensor("xf8", [P, kp * 2 * BC], FP8,
                             kind="ExternalInput").ap()
        wf8 = nc.dram_tensor("wf8", [P, kp * 2 * O], FP8,
                             kind="ExternalInput").ap()

    slabs = _slab_plan(cpg, prime, kc_total=kc2)

    with TileContext(nc) as tc:
        with (
            tc.tile_pool(name="stream", bufs=stream_bufs) as stream,
            tc.tile_pool(name="f8", bufs=2) as f8pool,
            tc.tile_pool(name="consts", bufs=1) as consts,
            tc.tile_pool(name="epi", bufs=1) as epi,
            tc.tile_pool(name="psum", bufs=2, space="PSUM") as psum,
            tc.tile_pool(name="psum1", bufs=1, space="PSUM") as psum1,
        ):
            ones_i = consts.tile([P, 1], I32, name="ones_i")
            nc.vector.memset(ones_i[:], 1)
            if with_bias:
                ones_f = consts.tile([1, P], F32, name="ones_f")
                nc.vector.memset(ones_f[:], 1.0)
                ones_row = consts.tile([1, P], BF16, name="ones_row")
                nc.vector.tensor_copy(out=ones_row[:], in_=ones_f[:])
                brow = consts.tile([1, O], BF16, name="brow")
                nc.sync.dma_start(out=brow[:], in_=bb_[None, :])
            coeff_b = consts.tile([P, 1], F32, name="coeff_b")
            nc.gpsimd.dma_start(out=coeff_b[:], in_=coeff[:])
            if wstat:
                # one-time [P, BC] broadcast of the per-row scales via the PE
                # (rep-invariant, so it lives outside the rep loop)
                ones1 = consts.tile([1, P], F32, name="ones1")
                nc.vector.memset(ones1[:], 1.0)
                srow_t = consts.tile([1, BC], F32, name="srow_t")
                nc.gpsimd.dma_start(out=srow_t[:], in_=srow[:])
                pss = psum1.tile([P, BC], F32, tag="pss", name="pss")
                nc.tensor.matmul(
                    pss[:], lhsT=ones1[:], rhs=srow_t[:],
                    start=True, stop=True,
                )
                s_bcast = consts.tile([P, BC], F32, name="s_bcast")
                nc.vector.tensor_copy(out=s_bcast[:], in_=pss[:])

            for _ in range(reps):
                if wstat:
                    psums = [
                        psum.tile([P, BC], F32, tag=f"ph{h}", name=f"ph{h}")
                        for h in range(O // P)
                    ]

                else:
                    psums = [
                        psum.tile([P, O], F32, tag=f"ps{m}", name=f"ps{m}")
                        for m in range(MB)
                    ]
                prep_t = epi.tile([P, 4 * MB], I32, tag="prep", name="prep_t")

                corrs = []

                def emit_prep(m):
                    # Entirely on GPSIMD (Pool): keeps the prep dependency
                    # chain out of the DVE/ACT in-order queues, which are
                    # busy casting the x stream.
                    idxt = prep_t[:, m:m + 1]
                    bpt = prep_t[:, MB + m:MB + m + 1]
                    g = prep_t[:, 2 * MB + m:2 * MB + m + 1].bitcast(F32)
                    s_m = prep_t[:, 3 * MB + m:3 * MB + m + 1].bitcast(F32)
                    # gather W[idx[i], :] rows (async SWDGE indirect DMA)
                    wg = epi.tile([P, O], BF16 if wstat else F32,
                                  tag=f"wg{m}", name=f"wg{m}")
                    nc.gpsimd.indirect_dma_start(
                        out=wg[:], out_offset=None,
                        in_=(wb16 if wstat else wf)[:],
                        in_offset=bass.IndirectOffsetOnAxis(
                            ap=idxt[:, :1], axis=0),
                    )
                    # u = coeff * (bitflip(g) - g); shift/xor are DVE-only
                    # (tiny [P,1] ops, prep landed long before -> no stall)
                    mask = epi.tile([P, 1], I32, tag=f"mask{m}", name=f"mask{m}")
                    nc.vector.tensor_scalar(
                        mask[:], ones_i[:], bpt[:, :1], None,
                        mybir.AluOpType.logical_shift_left,
                    )
                    gflip = epi.tile([P, 1], I32, tag=f"gflip{m}",
                                     name=f"gflip{m}")
                    nc.vector.tensor_tensor(
                        out=gflip[:], in0=g.bitcast(I32), in1=mask[:],
                        op=mybir.AluOpType.bitwise_xor,
                    )
                    u = epi.tile([P, 1], F32, tag=f"u{m}", name=f"u{m}")
                    nc.gpsimd.tensor_tensor(
                        out=u[:], in0=gflip[:].bitcast(F32), in1=g,
                        op=mybir.AluOpType.subtract,
                    )
                    nc.gpsimd.tensor_tensor(
                        out=u[:], in0=u[:], in1=coeff_b[:],
                        op=mybir.AluOpType.mult,
                    )
                    if wstat:
                        # diag(u) feeds a correction matmul into PSUM
                        diag_f = epi.tile([P, P], F32, tag=f"diagf{m}",
                                          name=f"diagf{m}")
                        nc.gpsimd.affine_select(
                            out=diag_f[:],
                            in_=u[:, :1].to_broadcast([P, P]),
                            pattern=[[-1, P]],
                            compare_op=mybir.AluOpType.is_equal,
                            fill=0.0,
                            base=0,
                            channel_multiplier=1,
                        )
                        diag = epi.tile([P, P], BF16, tag=f"diag{m}",
                                        name=f"diag{m}")
                        nc.gpsimd.tensor_copy(out=diag[:], in_=diag_f[:])
                        corrs.append((wg, diag))
                        return
                    corr = epi.tile([P, O], F32, tag=f"corr{m}",
                                    name=f"corr{m}")
                    nc.gpsimd.tensor_scalar(
                        corr[:], wg[:], u[:, :1], None,
                        mybir.AluOpType.mult
                    )
                    corrs.append((corr, s_m))

                # fp8 DoubleRow tail: DMAs issued mid-stream (data parks in
                # SBUF), matmuls run after the bf16 chunks so the fp8 burst
                # neither delays the ramp nor stalls the PE at the end
                f8_tiles = []

                def emit_f8_dma(sl0):
                    npair = min(4, kp - sl0)
                    x8s = f8pool.tile([P, npair * 2 * BC], FP8,
                                      tag=f"x8s{sl0}", name=f"x8s{sl0}",
                                      padded_shape=[P, 8 * BC])
                    nc.sync.dma_start(
                        out=x8s[:],
                        in_=xf8[:, sl0 * 2 * BC:(sl0 + npair) * 2 * BC])
                    w8s = f8pool.tile([P, npair * 2 * O], FP8,
                                      tag=f"w8s{sl0}", name=f"w8s{sl0}",
                                      padded_shape=[P, 8 * O])
                    nc.sync.dma_start(
                        out=w8s[:],
                        in_=wf8[:, sl0 * 2 * O:(sl0 + npair) * 2 * O])
                    f8_tiles.append((x8s, w8s, npair))

                f8_dma_at = {12 + 4 * i: sl0 for i, sl0 in
                             enumerate(range(0, kp, 4))}

                chunk_no = 0
                for k4, (k0, nch) in enumerate(slabs):
                    if k4 in f8_dma_at:
                        emit_f8_dma(f8_dma_at[k4])
                    xs = stream.tile([P, nch * BC], I8, tag="xs",
                                     name="xs", padded_shape=[P, cpg * BC])
                    ws = stream.tile([P, nch * O], BF16, tag="ws",
                                     name="ws", padded_shape=[P, cpg * O])
                    nc.sync.dma_start(
                        out=xs[:], in_=xq[:, k0 * BC:(k0 + nch) * BC])
                    nc.sync.dma_start(
                        out=ws[:], in_=wq[:, k0 * O:(k0 + nch) * O])
                    # int8 -> bf16 on-chip; DVE (2x mode) : ACT casts at 3:2
                    xsb = stream.tile([P, nch * BC], BF16, tag="xsb",
                                      name="xsb", padded_shape=[P, cpg * BC])
                    if k4 % cast_mod in act_slots:
                        nc.scalar.copy(out=xsb[:], in_=xs[:])
                    else:
                        nc.vector.tensor_copy(out=xsb[:], in_=xs[:])
                    if k4 == prep_dma_at:
                        # deferred so the first stream slabs win the DMA queue
                        nc.sync.dma_start(out=prep_t[:], in_=prep[:])
                    if k4 in prep_at:
                        # correction prep spread out behind the stream
                        emit_prep(prep_at.index(k4))
                    last_slab = k4 == len(slabs) - 1
                    for c in range(nch):
                        if wstat:
                            for h in range(O // P):
                                nc.tensor.matmul(
                                    psums[h][:],
                                    lhsT=ws[:, c * O + h * P:c * O + (h + 1) * P],
                                    rhs=xsb[:, c * BC:(c + 1) * BC],
                                    start=(chunk_no == 0),
                                    stop=False,
                                )
                        else:
                            for m in range(MB):
                                nc.tensor.matmul(
                                    psums[m][:],
                                    lhsT=xsb[:, c * BC + m * P:c * BC + (m + 1) * P],
                                    rhs=ws[:, c * O:(c + 1) * O],
                                    start=(chunk_no == 0),
                                    stop=(not with_bias and last_slab
                                          and c == nch - 1),
                                )
                        chunk_no += 1
                # flush any fp8 DMAs whose checkpoint slab didn't exist
                emitted = {s for k, s in f8_dma_at.items() if k < len(slabs)}
                for sl0 in range(0, kp, 4):
                    if sl0 not in emitted:
                        emit_f8_dma(sl0)
                # fp8 DoubleRow matmuls: 2 k-chunks per instruction
                for x8s, w8s, npair in f8_tiles:
                    for c in range(npair):
                        rhs8 = x8s[:, c * 2 * BC:(c + 1) * 2 * BC].rearrange(
                            "p (j n) -> p j n", j=2)
                        for h in range(O // P):
                            lhs8 = w8s[:, c * 2 * O + h * 2 * P:
                                       c * 2 * O + (h + 1) * 2 * P].rearrange(
                                "p (j m) -> p j m", j=2)
                            nc.tensor.matmul(
                                psums[h][:],
                                lhsT=lhs8,
                                rhs=rhs8,
                                start=False,
                                stop=False,
                                perf_mode=mybir.MatmulPerfMode.DoubleRow,
                            )
                for m in range(len(corrs), MB):
                    emit_prep(m)  # safety if the slab plan is very short
                if wstat:
                    # fold the correction into PSUM: one diag(u) matmul per
                    # (m-block, o-half); the last one closes each group
                    for m in range(MB):
                        wg, diag = corrs[m]
                        for h in range(O // P):
                            nc.tensor.matmul(
                                psums[h][:, m * P:(m + 1) * P],
                                lhsT=wg[:, h * P:(h + 1) * P],
                                rhs=diag[:],
                                start=False,
                                stop=(m == MB - 1),
                                skip_group_check=True,
                            )
                    for h in range(O // P):
                        outt = epi.tile([P, BC], F32, tag=f"outh{h}",
                                        name=f"outh{h}")
                        nc.vector.tensor_tensor(
                            out=outt[:], in0=psums[h][:], in1=s_bcast[:],
                            op=mybir.AluOpType.mult,
                        )
                        eng = nc.sync if h % 2 == 0 else nc.scalar
                        eng.dma_start(
                            out=out[h * P:(h + 1) * P, :], in_=outt[:])
                else:
                    if with_bias:
                        # bias: psum[m][i,:] += 1*b[:] (K=1 matmul ends group)
                        for m in range(MB):
                            nc.tensor.matmul(
                                psums[m][:],
                                lhsT=ones_row[:],
                                rhs=brow[:],
                                start=False,
                                stop=True,
                            )
                    for m in range(MB):
                        rows = slice(m * P, (m + 1) * P)
                        corr, s_m = corrs[m]
                        outt = epi.tile([P, O], F32, tag=f"outt{m}",
                                        name=f"outt{m}")
                        # out = psum * row_scale + correction, fused on DVE
                        nc.vector.scalar_tensor_tensor(
                            out=outt[:], in0=psums[m][:], scalar=s_m[:, :1],
                            in1=corr[:],
                            op0=mybir.AluOpType.mult, op1=mybir.AluOpType.add,
                        )
                        eng = nc.sync if m % 2 == 0 else nc.scalar
                        eng.dma_start(out=out[rows, :], in_=outt[:])

    _split_multi_waits(nc)
    return nc


_NC_CACHE = {}


def _get_nc(reps=1, with_bias=True):
    key = (reps, with_bias)
    if key not in _NC_CACHE:
        _NC_CACHE[key] = build(reps, with_bias=with_bias)
    return _NC_CACHE[key]


def make_in_maps(x, W, b, bitswap_coeff, idx, bit_positions):
    import ml_dtypes

    x = np.asarray(x, dtype=np.float32)
    Wf = np.ascontiguousarray(W, dtype=np.float32)
    b = np.ascontiguousarray(b, dtype=np.float32)
    coeff = np.full((P, 1), np.asarray(bitswap_coeff, dtype=np.float32))
    idx = np.asarray(idx, dtype=np.int32)
    bpos = np.asarray(bit_positions, dtype=np.int32)

    # symmetric per-row int8 quantization of x
    s = np.abs(x).max(axis=1) / 127.0
    s = np.maximum(s, 1e-30).astype(np.float32)
    xq8 = np.rint(x / s[:, None]).clip(-127, 127).astype(np.int8)
    g_all = x[np.arange(B), idx].astype(np.float32)

    kp = FP8_PAIRS
    kc2f = (KC - 2 * kp) * P  # feature boundary of the fp8 tail region
    if kp:
        e4 = ml_dtypes.float8_e4m3
        # x tail in e4m3; the 1/64 here cancels the 64x on W8 so the fp8
        # product lands in the same units as the int8 part -> one PSUM
        x8 = (x[:, kc2f:] / (FP8_WSCALE * s[:, None])).astype(e4)
        # W tail pre-scaled by 64 to stay in e4m3 normal range
        W8 = (FP8_WSCALE * Wf[kc2f:, :]).astype(e4)
        # [c, j, p, h, m] -> [p, c, h, j, m]
        wf8 = np.ascontiguousarray(
            W8.reshape(kp, 2, P, 2, P).transpose(2, 0, 3, 1, 4)
            .reshape(P, kp * 2 * O)
        )

    # W in bf16, flat [P, KC*O] layout: wq[p, k*O + o] = W[k*P + p, o]
    wb16 = np.ascontiguousarray(Wf.astype(ml_dtypes.bfloat16))
    wq = np.ascontiguousarray(
        wb16.reshape(KC, P, O).transpose(1, 0, 2).reshape(P, KC * O)
    )
    bmm = b.astype(ml_dtypes.bfloat16)

    in_maps = []
    for c in range(N_CORES):
        rows = slice(c * BC, (c + 1) * BC)
        # x slice in flat [P, KC*BC] layout: xqc[p, k*BC + i] = xq8[i0+i, k*P+p]
        xqc = np.ascontiguousarray(
            xq8[rows].reshape(BC, KC, P).transpose(2, 1, 0).reshape(P, KC * BC)
        )
        # packed [P, 4*MB] per-row scalars; [P, m] column = rows m*P..(m+1)*P
        packed = np.concatenate(
            [
                idx[rows].reshape(MB, P).T,
                bpos[rows].reshape(MB, P).T,
                g_all[rows].view(np.int32).reshape(MB, P).T,
                s[rows].view(np.int32).reshape(MB, P).T,
            ],
            axis=1,
        ).astype(np.int32)
        m = {
            "xq": xqc,
            "wq": wq,
            "wf": Wf,
            "prep": np.ascontiguousarray(packed),
            "b": bmm,
            "coeff": coeff,
            "srow": np.ascontiguousarray(s[rows])[None, :],
            "wb16": wb16,
        }
        if kp:
            # x8 core slice [BC, kp*2*P] -> [n, c, j, p] -> [p, c, j, n]
            m["xf8"] = np.ascontiguousarray(
                x8[rows].reshape(BC, kp, 2, P).transpose(3, 1, 2, 0)
                .reshape(P, kp * 2 * BC)
            )
            m["wf8"] = wf8
        in_maps.append(m)
    return in_maps


def kernel(x, W, b, bitswap_coeff, idx, bit_positions):
    with_bias = bool(np.any(np.asarray(b)))
    nc = _get_nc(with_bias=with_bias)
    in_maps = make_in_maps(x, W, b, bitswap_coeff, idx, bit_positions)
    res = run_bass_kernel_spmd(nc, in_maps, core_ids=list(range(N_CORES)))
    outs = [res.results[c]["out"] for c in range(N_CORES)]
    if WSTAT and not with_bias:
        outs = [o.T for o in outs]
    return np.concatenate(outs, axis=0)


# revision 50
# speedup vs baseline: 1.2032x; 1.2032x over previous
"""Trainium2 Bass kernel for nn_BitSwapWrapper.

Reference computation:
    g    = x[rows, idx]                       # one gathered element per row
    u    = coeff * (bitflip(g, bit_pos) - g)
    pert = scatter(zeros_like(x), (rows, idx), u)
    out  = (x + pert) @ W + b

Because pert has exactly one nonzero per row, (x + pert) @ W decomposes as
    out[i, :] = (x @ W)[i, :] + u[i] * W[idx[i], :] + b
so no [B, F] scatter tensor is ever materialized: the kernel streams x
through a K-accumulated matmul and applies the rank-per-row correction with
an indirect-DMA gather of the needed W rows (u is formed on-chip from the
exact fp32 gathered values + bit positions).

Distribution: data-parallel over the batch dim across 8 NeuronCores
(x/idx/bit_positions sharded on dim 0, W/b/coeff replicated), per the
sharding hint. Each core computes its [512, 256] slice of the output.

Precision/traffic design (memory-regime problem, PE sets the floor):
  - x is streamed as int8 with per-batch-row scales (symmetric absmax
    quantization done on host). 8 MB/core instead of 32 MB fp32.
  - W is streamed as bf16 (8 MB/core).
  - int8 x is converted to bf16 on-chip (exact: |q| <= 127), with the
    conversion alternated between the DVE and Activation engines so it
    stays off the critical path; the PE then runs bf16 x bf16 matmuls
    (~55 us of PE work, the binding resource -- DMA is ~46 us).
  - WSTAT form: W-half tiles are the stationary operand and x chunks the
    moving one (N=512), halving the PE instruction count vs the x-stationary
    form; measured ~3.5 us faster on HW (per-instruction overhead the cost
    model does not carry). Output leaves as [O, BC], transposed on host.
  - the bit-flip correction is folded into PSUM with one diag(u) matmul per
    (row-block, O-half); per-row scales apply in the epilogue as one
    [P, BC] multiply against a PE-broadcast scale tile (built once, outside
    the rep loop, since it is rep-invariant).
  - the correction uses exact fp32 gathered values (gh), so the output
    error is x-quantization + W rounding only.
  - hybrid K-split: the last 2*FP8_PAIRS k-chunks run as fp8e4 DoubleRow
    matmuls (2 chunks per instruction at 0.5 cycles/row), cutting the PE
    floor from 54.6 us to 48.2 us at FP8_PAIRS=10. x is e4m3(x/(64 s))
    and W is e4m3(64 W) there, so products land in the same units as the
    int8 part and accumulate into the same PSUM group. The fp8 DMAs are
    issued mid-stream (parked in SBUF) and the matmuls run after the bf16
    chunks, so the burst neither delays the ramp nor stalls the PE.
    End-to-end error 1.65e-2 vs the 2e-2 gate, deterministic (fixed
    inputs), HW-verified (numpy prediction matched HW to ~1e-5).
"""

import numpy as np

import concourse.bass as bass
import concourse.mybir as mybir
from concourse.bass_utils import run_bass_kernel_spmd
from concourse.tile import TileContext

N_CORES = 8
B, F, O = 4096, 16384, 256
BC = B // N_CORES        # 512 batch rows per core
P = 128
KC = F // P              # 128 contraction chunks
MB = BC // P             # 4 output row-blocks per core

F32 = mybir.dt.float32
BF16 = mybir.dt.bfloat16
I32 = mybir.dt.int32
I8 = mybir.dt.int8
FP8 = mybir.dt.float8e4  # = ml_dtypes.float8_e4m3 on host

# Hybrid K-split: the last 2*FP8_PAIRS k-chunks run as fp8 DoubleRow matmuls
# (2 chunks per matmul at 0.5 cycles/row -> ~5.1 us less PE work per rep).
# Error budget: int8-x part ~9.6e-3, fp8 part adds in quadrature ->
# 1.65e-2 measured end-to-end on the real inputs (gate 2e-2). W in the fp8
# region is pre-scaled by 64 and x by 1/64 (keeps e4m3 out of subnormals
# while the product stays in the int8-part units -> same PSUM group).
FP8_PAIRS = 10
FP8_WSCALE = 64.0


def _split_multi_waits(nc):
    """This container's walrus build rejects more than one sync-wait command
    per instruction; split extras onto single-wait NOPs on the same engine."""
    cur_bb = nc.cur_bb.bb
    for f in nc.m.functions:
        for bb in f.blocks:
            il = bb.instructions
            i = 0
            while i < len(il):
                ins = il[i]
                si = getattr(ins, "sync_info", None)
                if si is not None and si.on_wait and len(si.on_wait) > 1:
                    waits = list(si.on_wait)
                    extra, keep = waits[:-1], waits[-1:]
                    carriers = []
                    for w in extra:
                        nop = nc.engines[ins.engine].nop(nofuse=True).ins
                        tail = cur_bb.instructions.pop()
                        assert tail is nop
                        nop.sync_info = mybir.SyncInfo(on_wait=[w], on_update=[])
                        carriers.append(nop)
                    ins.sync_info = mybir.SyncInfo(
                        on_wait=keep, on_update=list(si.on_update or [])
                    )
                    il[i:i] = carriers
                    i += len(carriers)
                i += 1


def _slab_plan(cpg, prime, kc_total=KC):
    """Chunk slabs: a few small ones first to prime the PE pipeline, then
    full-size slabs. Returns [(k0, nchunks), ...] covering kc_total chunks."""
    slabs = []
    k = 0
    for n in prime:
        slabs.append((k, n))
        k += n
    while k < kc_total:
        n = min(cpg, kc_total - k)
        slabs.append((k, n))
        k += n
    return slabs


WSTAT = True  # stationary-W matmul form: half the PE instructions (N=512)


def build(reps=1, stream_bufs=12, cpg=4, prime=(1, 1, 2), with_bias=True,
          act_slots=(1,), cast_mod=2, prep_at=(6, 9, 12, 15), prep_dma_at=2,
          wstat=None, fp8_pairs=None):
    if wstat is None:
        wstat = WSTAT
    if wstat and with_bias:
        wstat = False  # wstat path assumes b == 0
    if fp8_pairs is None:
        fp8_pairs = FP8_PAIRS
    if not wstat:
        fp8_pairs = 0
    kp = fp8_pairs
    kc2 = KC - 2 * kp  # chunks handled by the int8->bf16 stream
    nc = bass.Bass("TRN2", target_bir_lowering=False, debug=False)
    xq = nc.dram_tensor("xq", [P, KC * BC], I8, kind="ExternalInput").ap()
    wq = nc.dram_tensor("wq", [P, KC * O], BF16, kind="ExternalInput").ap()
    wf = nc.dram_tensor("wf", [F, O], F32, kind="ExternalInput").ap()
    # packed per-row scalars: [idx | bpos | gh bits | scale bits], MB cols each
    prep = nc.dram_tensor("prep", [P, 4 * MB], I32, kind="ExternalInput").ap()
    bb_ = nc.dram_tensor("b", [O], BF16, kind="ExternalInput").ap()
    coeff = nc.dram_tensor("coeff", [P, 1], F32, kind="ExternalInput").ap()
    if wstat:
        srow = nc.dram_tensor("srow", [1, BC], F32, kind="ExternalInput").ap()
        # row-major bf16 W for the correction gather: bf16 operands keep the
        # diag(u) matmuls at 1 cycle/row (f32 costs 4x on the PE)
        wb16 = nc.dram_tensor("wb16", [F, O], BF16, kind="ExternalInput").ap()
        out = nc.dram_tensor("out", [O, BC], F32, kind="ExternalOutput").ap()
    else:
        out = nc.dram_tensor("out", [BC, O], F32, kind="ExternalOutput").ap()
    if kp:
        # fp8 tail region, plane-major pair layouts for DoubleRow:
        # xf8[p, c*2*BC + j*BC + n] = e4m3(x[n, (kc2+2c+j)*P + p] / s[n])
        # wf8[p, c*4*P*? ...]: [c, h, j, m] -> W8[(kc2+2c+j)*P + p, h*P+m]
        xf8 = nc.dram_tensor("xf8", [P, kp * 2 * BC], FP8,
                             kind="ExternalInput").ap()
        wf8 = nc.dram_tensor("wf8", [P, kp * 2 * O], FP8,
                             kind="ExternalInput").ap()

    slabs = _slab_plan(cpg, prime, kc_total=kc2)

    with TileContext(nc) as tc:
        with (
            tc.tile_pool(name="stream", bufs=stream_bufs) as stream,
            tc.tile_pool(name="f8", bufs=2) as f8pool,
            tc.tile_pool(name="consts", bufs=1) as consts,
            tc.tile_pool(name="epi", bufs=1) as epi,
            tc.tile_pool(name="psum", bufs=2, space="PSUM") as psum,
            tc.tile_pool(name="psum1", bufs=1, space="PSUM") as psum1,
        ):
            ones_i = consts.tile([P, 1], I32, name="ones_i")
            nc.vector.memset(ones_i[:], 1)
            if with_bias:
                ones_f = consts.tile([1, P], F32, name="ones_f")
                nc.vector.memset(ones_f[:], 1.0)
                ones_row = consts.tile([1, P], BF16, name="ones_row")
                nc.vector.tensor_copy(out=ones_row[:], in_=ones_f[:])
                brow = consts.tile([1, O], BF16, name="brow")
                nc.sync.dma_start(out=brow[:], in_=bb_[None, :])
            coeff_b = consts.tile([P, 1], F32, name="coeff_b")
            nc.gpsimd.dma_start(out=coeff_b[:], in_=coeff[:])
            if wstat:
                # one-time [P, BC] broadcast of the per-row scales via the PE
                # (rep-invariant, so it lives outside the rep loop)
                ones1 = consts.tile([1, P], F32, name="ones1")
                nc.vector.memset(ones1[:], 1.0)
                srow_t = consts.tile([1, BC], F32, name="srow_t")
                nc.gpsimd.dma_start(out=srow_t[:], in_=srow[:])
                pss = psum1.tile([P, BC], F32, tag="pss", name="pss")
                nc.tensor.matmul(
                    pss[:], lhsT=ones1[:], rhs=srow_t[:],
                    start=True, stop=True,
                )
                s_bcast = consts.tile([P, BC], F32, name="s_bcast")
                nc.vector.tensor_copy(out=s_bcast[:], in_=pss[:])

            for _ in range(reps):
                if wstat:
                    psums = [
                        psum.tile([P, BC], F32, tag=f"ph{h}", name=f"ph{h}")
                        for h in range(O // P)
                    ]

                else:
                    psums = [
                        psum.tile([P, O], F32, tag=f"ps{m}", name=f"ps{m}")
                        for m in range(MB)
                    ]
                prep_t = epi.tile([P, 4 * MB], I32, tag="prep", name="prep_t")

                corrs = []

                def emit_prep(m):
                    # Entirely on GPSIMD (Pool): keeps the prep dependency
                    # chain out of the DVE/ACT in-order queues, which are
                    # busy casting the x stream.
                    idxt = prep_t[:, m:m + 1]
                    bpt = prep_t[:, MB + m:MB + m + 1]
                    g = prep_t[:, 2 * MB + m:2 * MB + m + 1].bitcast(F32)
                    s_m = prep_t[:, 3 * MB + m:3 * MB + m + 1].bitcast(F32)
                    # gather W[idx[i], :] rows (async SWDGE indirect DMA)
                    wg = epi.tile([P, O], BF16 if wstat else F32,
                                  tag=f"wg{m}", name=f"wg{m}")
                    nc.gpsimd.indirect_dma_start(
                        out=wg[:], out_offset=None,
                        in_=(wb16 if wstat else wf)[:],
                        in_offset=bass.IndirectOffsetOnAxis(
                            ap=idxt[:, :1], axis=0),
                    )
                    # u = coeff * (bitflip(g) - g); shift/xor are DVE-only
                    # (tiny [P,1] ops, prep landed long before -> no stall)
                    mask = epi.tile([P, 1], I32, tag=f"mask{m}", name=f"mask{m}")
                    nc.vector.tensor_scalar(
                        mask[:], ones_i[:], bpt[:, :1], None,
                        mybir.AluOpType.logical_shift_left,
                    )
                    gflip = epi.tile([P, 1], I32, tag=f"gflip{m}",
                                     name=f"gflip{m}")
                    nc.vector.tensor_tensor(
                        out=gflip[:], in0=g.bitcast(I32), in1=mask[:],
                        op=mybir.AluOpType.bitwise_xor,
                    )
                    u = epi.tile([P, 1], F32, tag=f"u{m}", name=f"u{m}")
                    nc.gpsimd.tensor_tensor(
                        out=u[:], in0=gflip[:].bitcast(F32), in1=g,
                        op=mybir.AluOpType.subtract,
                    )
                    nc.gpsimd.tensor_tensor(
                        out=u[:], in0=u[:], in1=coeff_b[:],
                        op=mybir.AluOpType.mult,
                    )
                    if wstat:
                        # diag(u) feeds a correction matmul into PSUM
                        diag_f = epi.tile([P, P], F32, tag=f"diagf{m}",
                                          name=f"diagf{m}")
                        nc.gpsimd.affine_select(
                            out=diag_f[:],
                            in_=u[:, :1].to_broadcast([P, P]),
                            pattern=[[-1, P]],
                            compare_op=mybir.AluOpType.is_equal,
                            fill=0.0,
                            base=0,
                            channel_multiplier=1,
                        )
                        diag = epi.tile([P, P], BF16, tag=f"diag{m}",
                                        name=f"diag{m}")
                        nc.gpsimd.tensor_copy(out=diag[:], in_=diag_f[:])
                        corrs.append((wg, diag))
                        return
                    corr = epi.tile([P, O], F32, tag=f"corr{m}",
                                    name=f"corr{m}")
                    nc.gpsimd.tensor_scalar(
                        corr[:], wg[:], u[:, :1], None,
                        mybir.AluOpType.mult
                    )
                    corrs.append((corr, s_m))

                # fp8 DoubleRow tail: DMAs issued mid-stream (data parks in
                # SBUF), matmuls run after the bf16 chunks so the fp8 burst
                # neither delays the ramp nor stalls the PE at the end
                f8_tiles = []

                def emit_f8_dma(sl0):
                    npair = min(4, kp - sl0)
                    x8s = f8pool.tile([P, npair * 2 * BC], FP8,
                                      tag=f"x8s{sl0}", name=f"x8s{sl0}",
                                      padded_shape=[P, 8 * BC])
                    nc.sync.dma_start(
                        out=x8s[:],
                        in_=xf8[:, sl0 * 2 * BC:(sl0 + npair) * 2 * BC])
                    w8s = f8pool.tile([P, npair * 2 * O], FP8,
                                      tag=f"w8s{sl0}", name=f"w8s{sl0}",
                                      padded_shape=[P, 8 * O])
                    nc.sync.dma_start(
                        out=w8s[:],
                        in_=wf8[:, sl0 * 2 * O:(sl0 + npair) * 2 * O])
                    f8_tiles.append((x8s, w8s, npair))

                # inject late + spaced: the stream banks ~0.29us of DMA slack
                # per slab, so each ~2us fp8 burst needs ~5 slabs of headroom
                f8_dma_at = {16 + 5 * i: sl0 for i, sl0 in
                             enumerate(range(0, kp, 4))}

                chunk_no = 0
                for k4, (k0, nch) in enumerate(slabs):
                    if k4 in f8_dma_at:
                        emit_f8_dma(f8_dma_at[k4])
                    xs = stream.tile([P, nch * BC], I8, tag="xs",
                                     name="xs", padded_shape=[P, cpg * BC])
                    ws = stream.tile([P, nch * O], BF16, tag="ws",
                                     name="ws", padded_shape=[P, cpg * O])
                    nc.sync.dma_start(
                        out=xs[:], in_=xq[:, k0 * BC:(k0 + nch) * BC])
                    nc.sync.dma_start(
                        out=ws[:], in_=wq[:, k0 * O:(k0 + nch) * O])
                    # int8 -> bf16 on-chip; DVE (2x mode) : ACT casts at 3:2
                    xsb = stream.tile([P, nch * BC], BF16, tag="xsb",
                                      name="xsb", padded_shape=[P, cpg * BC])
                    if k4 % cast_mod in act_slots:
                        nc.scalar.copy(out=xsb[:], in_=xs[:])
                    else:
                        nc.vector.tensor_copy(out=xsb[:], in_=xs[:])
                    if k4 == prep_dma_at:
                        # deferred so the first stream slabs win the DMA queue
                        nc.sync.dma_start(out=prep_t[:], in_=prep[:])
                    if k4 in prep_at:
                        # correction prep spread out behind the stream
                        emit_prep(prep_at.index(k4))
                    last_slab = k4 == len(slabs) - 1
                    for c in range(nch):
                        if wstat:
                            for h in range(O // P):
                                nc.tensor.matmul(
                                    psums[h][:],
                                    lhsT=ws[:, c * O + h * P:c * O + (h + 1) * P],
                                    rhs=xsb[:, c * BC:(c + 1) * BC],
                                    start=(chunk_no == 0),
                                    stop=False,
                                )
                        else:
                            for m in range(MB):
                                nc.tensor.matmul(
                                    psums[m][:],
                                    lhsT=xsb[:, c * BC + m * P:c * BC + (m + 1) * P],
                                    rhs=ws[:, c * O:(c + 1) * O],
                                    start=(chunk_no == 0),
                                    stop=(not with_bias and last_slab
                                          and c == nch - 1),
                                )
                        chunk_no += 1
                # flush any fp8 DMAs whose checkpoint slab didn't exist
                emitted = {s for k, s in f8_dma_at.items() if k < len(slabs)}
                for sl0 in range(0, kp, 4):
                    if sl0 not in emitted:
                        emit_f8_dma(sl0)
                # fp8 DoubleRow matmuls: 2 k-chunks per instruction
                for x8s, w8s, npair in f8_tiles:
                    for c in range(npair):
                        rhs8 = x8s[:, c * 2 * BC:(c + 1) * 2 * BC].rearrange(
                            "p (j n) -> p j n", j=2)
                        for h in range(O // P):
                            lhs8 = w8s[:, c * 2 * O + h * 2 * P:
                                       c * 2 * O + (h + 1) * 2 * P].rearrange(
                                "p (j m) -> p j m", j=2)
                            nc.tensor.matmul(
                                psums[h][:],
                                lhsT=lhs8,
                                rhs=rhs8,
                                start=False,
                                stop=False,
                                perf_mode=mybir.MatmulPerfMode.DoubleRow,
                            )
                for m in range(len(corrs), MB):
                    emit_prep(m)  # safety if the slab plan is very short
                if wstat:
                    # fold the correction into PSUM: one diag(u) matmul per
                    # (m-block, o-half); the last one closes each group
                    for m in range(MB):
                        wg, diag = corrs[m]
                        for h in range(O // P):
                            nc.tensor.matmul(
                                psums[h][:, m * P:(m + 1) * P],
                                lhsT=wg[:, h * P:(h + 1) * P],
                                rhs=diag[:],
                                start=False,
                                stop=(m == MB - 1),
                                skip_group_check=True,
                            )
                    for h in range(O // P):
                        outt = epi.tile([P, BC], F32, tag=f"outh{h}",
                                        name=f"outh{h}")
                        nc.vector.tensor_tensor(
                            out=outt[:], in0=psums[h][:], in1=s_bcast[:],
                            op=mybir.AluOpType.mult,
                        )
                        eng = nc.sync if h % 2 == 0 else nc.scalar
                        eng.dma_start(
                            out=out[h * P:(h + 1) * P, :], in_=outt[:])
                else:
                    if with_bias:
                        # bias: psum[m][i,:] += 1*b[:] (K=1 matmul ends group)
                        for m in range(MB):
                            nc.tensor.matmul(
                                psums[m][:],
                                lhsT=ones_row[:],
                                rhs=brow[:],
                                start=False,
                                stop=True,
                            )
                    for m in range(MB):
                        rows = slice(m * P, (m + 1) * P)
                        corr, s_m = corrs[m]
                        outt = epi.tile([P, O], F32, tag=f"outt{m}",
                                        name=f"outt{m}")
                        # out = psum * row_scale + correction, fused on DVE
                        nc.vector.scalar_tensor_tensor(
                            out=outt[:], in0=psums[m][:], scalar=s_m[:, :1],
                            in1=corr[:],
                            op0=mybir.AluOpType.mult, op1=mybir.AluOpType.add,
                        )
                        eng = nc.sync if m % 2 == 0 else nc.scalar
                        eng.dma_start(out=out[rows, :], in_=outt[:])

    _split_multi_waits(nc)
    return nc


_NC_CACHE = {}


def _get_nc(reps=1, with_bias=True):
    key = (reps, with_bias)
    if key not in _NC_CACHE:
        _NC_CACHE[key] = build(reps, with_bias=with_bias)
    return _NC_CACHE[key]


def make_in_maps(x, W, b, bitswap_coeff, idx, bit_positions):
    import ml_dtypes

    x = np.asarray(x, dtype=np.float32)
    Wf = np.ascontiguousarray(W, dtype=np.float32)
    b = np.ascontiguousarray(b, dtype=np.float32)
    coeff = np.full((P, 1), np.asarray(bitswap_coeff, dtype=np.float32))
    idx = np.asarray(idx, dtype=np.int32)
    bpos = np.asarray(bit_positions, dtype=np.int32)

    # symmetric per-row int8 quantization of x
    s = np.abs(x).max(axis=1) / 127.0
    s = np.maximum(s, 1e-30).astype(np.float32)
    xq8 = np.rint(x / s[:, None]).clip(-127, 127).astype(np.int8)
    g_all = x[np.arange(B), idx].astype(np.float32)

    kp = FP8_PAIRS
    kc2f = (KC - 2 * kp) * P  # feature boundary of the fp8 tail region
    if kp:
        e4 = ml_dtypes.float8_e4m3
        # x tail in e4m3; the 1/64 here cancels the 64x on W8 so the fp8
        # product lands in the same units as the int8 part -> one PSUM
        x8 = (x[:, kc2f:] / (FP8_WSCALE * s[:, None])).astype(e4)
        # W tail pre-scaled by 64 to stay in e4m3 normal range
        W8 = (FP8_WSCALE * Wf[kc2f:, :]).astype(e4)
        # [c, j, p, h, m] -> [p, c, h, j, m]
        wf8 = np.ascontiguousarray(
            W8.reshape(kp, 2, P, 2, P).transpose(2, 0, 3, 1, 4)
            .reshape(P, kp * 2 * O)
        )

    # W in bf16, flat [P, KC*O] layout: wq[p, k*O + o] = W[k*P + p, o]
    wb16 = np.ascontiguousarray(Wf.astype(ml_dtypes.bfloat16))
    wq = np.ascontiguousarray(
        wb16.reshape(KC, P, O).transpose(1, 0, 2).reshape(P, KC * O)
    )
    bmm = b.astype(ml_dtypes.bfloat16)

    in_maps = []
    for c in range(N_CORES):
        rows = slice(c * BC, (c + 1) * BC)
        # x slice in flat [P, KC*BC] layout: xqc[p, k*BC + i] = xq8[i0+i, k*P+p]
        xqc = np.ascontiguousarray(
            xq8[rows].reshape(BC, KC, P).transpose(2, 1, 0).reshape(P, KC * BC)
        )
        # packed [P, 4*MB] per-row scalars; [P, m] column = rows m*P..(m+1)*P
        packed = np.concatenate(
            [
                idx[rows].reshape(MB, P).T,
                bpos[rows].reshape(MB, P).T,
                g_all[rows].view(np.int32).reshape(MB, P).T,
                s[rows].view(np.int32).reshape(MB, P).T,
            ],
            axis=1,
        ).astype(np.int32)
        m = {
            "xq": xqc,
            "wq": wq,
            "wf": Wf,
            "prep": np.ascontiguousarray(packed),
            "b": bmm,
            "coeff": coeff,
            "srow": np.ascontiguousarray(s[rows])[None, :],
            "wb16": wb16,
        }
        if kp:
            # x8 core slice [BC, kp*2*P] -> [n, c, j, p] -> [p, c, j, n]
            m["xf8"] = np.ascontiguousarray(
                x8[rows].reshape(BC, kp, 2, P).transpose(3, 1, 2, 0)
                .reshape(P, kp * 2 * BC)
            )
            m["wf8"] = wf8
        in_maps.append(m)
    return in_maps


def kernel(x, W, b, bitswap_coeff, idx, bit_positions):
    with_bias = bool(np.any(np.asarray(b)))
    nc = _get_nc(with_bias=with_bias)
    in_maps = make_in_maps(x, W, b, bitswap_coeff, idx, bit_positions)
    res = run_bass_kernel_spmd(nc, in_maps, core_ids=list(range(N_CORES)))
    outs = [res.results[c]["out"] for c in range(N_CORES)]
    if WSTAT and not with_bias:
        outs = [o.T for o in outs]
    return np.concatenate(outs, axis=0)
